# revision 17
# baseline (speedup 1.0000x reference)
"""DeepWalk hierarchical-softmax loss kernel for Trainium2 (8 NeuronCores).

Computation (per the nn.Module reference):
    ctx, leaf = edges[:, 0], edges[:, 1]
    x[e, l] = dot(Z[ctx[e]], Z[path_nodes[leaf[e], l]])
    loss = sum(path_mask[leaf] * softplus(-path_signs[leaf] * x))

Sharding: data-parallel over the edge batch; 8 cores x 4096 edges. Z is
cast to bf16 on the host and replicated; all embedding movement and all
float math happen on device.

Device-side algorithm per core (slot = one (edge, level) pair):
  - the host sorts each core's 81920 slots by node id and buckets them into
    int16-addressable 32768-row chunks of Z, so the 640 per-row indirect
    DMAs of the old design become ~45 bulk dma_gather instructions
    (SWDGE descriptor-gen fixed cost ~1us/instruction was the bottleneck:
    768 x 1.4us = 1.1ms of serialized GPSIMD time in the baseline).
  - zv = Z[ctx] lands token-major in SBUF via per-ctx-chunk dma_gather
    (edges pre-sorted by ctx chunk); zpT = Z[node] per stage via
    transposed dma_gather ([d on partitions, slot on free axis]).
  - zvT_dup pairs each slot with its edge's zv row via SBUF-source
    dma_gather (token ids are int16, order-free).
  - prod = zpT * zvT_dup on DVE (bf16); per-512-slot column sums via
    one-hot-column matmul accumulation into PSUM rows (slot s -> PSUM
    [row, col]); masked softplus epilogue reads PSUM, emits [128, 1]
    partial sums per core; the host adds them up.

HW-probed notes:
  - dma_gather idx tensors are int16, wrapped i%16 x i//16 over 16
    partitions AND replicated across all 8 GPSIMD core groups (the sim
    reads only partitions 0-15; HW core k reads partitions 16k..16k+15).
  - engine APs must start at partition 0/32/64/96 -> per-row matmul
    outputs are not possible; use one-hot lhsT accumulation instead.
  - plain tensor_scalar hangs this runtime; use scalar_tensor_tensor with
    op1=bypass (baseline-probed).
"""

import dataclasses
import os
import tempfile

# The neuronx-cc on-disk compile cache keys on the HLO graph hash, which does
# NOT include the bass_exec backend_config (the embedded BIR). Two different
# kernel builds with the same I/O signature therefore collide, and a stale
# NEFF from an earlier build would silently run instead of this one. Use a
# fresh per-process cache dir, set before libneuronxla reads the env.
os.environ.setdefault(
    "NEURON_COMPILE_CACHE_URL", tempfile.mkdtemp(prefix="neuron_cc_cache_")
)

import numpy as np
import ml_dtypes

import concourse.bacc as bacc
import concourse.bass as bass
import concourse.mybir as mybir
import concourse.tile as tile
from concourse import library_config
from concourse.bass_utils import run_bass_kernel_spmd

P = 128
BF16 = ml_dtypes.bfloat16
N_SWDGE_QUEUES = max(1, min(4, int(os.environ.get("DW_QUEUES", "4"))))


@dataclasses.dataclass(frozen=True)
class Shape:
    n_cores: int = 8
    epc: int = 4096            # edges per core
    depth: int = 20            # L
    dim: int = 128             # D
    nz: int = 999_999          # Z rows
    chunk: int = 32_768        # int16-addressable rows per dma_gather call
    stage: int = 4096          # slots per stage (8 matmuls x 512)
    bank: int = 512            # PSUM bank cols (f32)
    sub: int = 896             # max idxs per dma_gather call: 896/16+2 = 58
                               # descriptors, under the 64-desc packet cap
                               # (single_packet=True wedges HW above it)


@dataclasses.dataclass(frozen=True)
class Plan:
    """Static (compile-time) structure shared by all cores."""
    shape: Shape
    e_pad: int                                  # padded token count
    zv_calls: tuple                             # ((chunk, tok_off, cnt), ...)
    zp_calls: tuple                             # per stage: ((chunk, cnt), ...)
    n_stages: int
    n_banks: int
    s_tot: int                                  # padded slot count (= n_stages*stage)


def _wrap_idx(idx: np.ndarray) -> np.ndarray:
    """[n] int16 -> [128, n/16] wrapped + replicated across 8 core groups."""
    n = len(idx)
    assert n % 16 == 0
    out = np.zeros((16, n // 16), dtype=np.int16)
    out[np.arange(n) % 16, np.arange(n) // 16] = idx
    return np.ascontiguousarray(np.tile(out, (8, 1)))


def _rup(x, m):
    return (x + m - 1) // m * m


def make_plan_and_maps(edges, path_nodes, path_signs, path_mask, Z, sh=None):
    """Host prep: bf16 Z, chunk-sorted index plans, per-core input maps."""
    if sh is None:
        sh = Shape()
    edges = np.asarray(edges)
    b = edges.shape[0]
    assert b == sh.n_cores * sh.epc, (b, sh)
    pn = np.asarray(path_nodes)
    sm_tab = (np.asarray(path_signs) * np.asarray(path_mask)).astype(np.float32)
    mk_tab = np.asarray(path_mask).astype(np.float32)
    z = np.asarray(Z)
    nz = z.shape[0]
    assert nz == sh.nz and z.shape[1] == sh.dim and pn.shape[1] == sh.depth
    zb = z.astype(BF16)

    n_zchunks = (nz + sh.chunk - 1) // sh.chunk
    L, epc = sh.depth, sh.epc

    # ---- per-core raw data
    cores = []
    for c in range(sh.n_cores):
        e = edges[c * sh.epc : (c + 1) * sh.epc]
        ctx, leaf = e[:, 0].astype(np.int64), e[:, 1].astype(np.int64)
        nodes = pn[leaf].astype(np.int64)          # [epc, L]
        sm = sm_tab[leaf]                          # [epc, L]
        mk = mk_tab[leaf]
        cores.append((ctx, leaf, nodes, sm, mk))

    # ---- zv token plan: edges sorted by ctx; per-ctx-chunk counts padded to
    # x128 and maxed across cores (SPMD: one static program for all cores).
    ctx_chunk_cnt = np.zeros((sh.n_cores, n_zchunks), np.int64)
    eords = []
    for c, (ctx, *_ ) in enumerate(cores):
        eord = np.argsort(ctx, kind="stable")
        eords.append(eord)
        cc = ctx // sh.chunk
        np.add.at(ctx_chunk_cnt[c], cc, 1)
    zv_cnt = np.array([_rup(int(ctx_chunk_cnt[:, ch].max()), P) for ch in range(n_zchunks)])
    zv_off = np.concatenate([[0], np.cumsum(zv_cnt)])
    e_pad = int(zv_off[-1])
    def pieces(cnt):
        out = []
        while cnt > 0:
            take = min(cnt, sh.sub)
            out.append(take)
            cnt -= take
        return out

    zv_calls = tuple(
        (ch, int(zv_off[ch]) + delta, take)
        for ch in range(n_zchunks) if zv_cnt[ch] > 0
        for delta, take in zip(
            np.concatenate([[0], np.cumsum(pieces(int(zv_cnt[ch])))]),
            pieces(int(zv_cnt[ch])),
        )
    )

    # ---- zp slot plan: slots sorted by node id; per-node-chunk counts
    # padded to x128, maxed across cores; then split into fixed-size stages.
    node_chunk_cnt = np.zeros((sh.n_cores, n_zchunks), np.int64)
    for c, (_, _, nodes, _, _) in enumerate(cores):
        nc_ = nodes.reshape(-1) // sh.chunk
        np.add.at(node_chunk_cnt[c], nc_, 1)
    zp_cnt = np.array([_rup(int(node_chunk_cnt[:, ch].max()), P) for ch in range(n_zchunks)])
    zp_off = np.concatenate([[0], np.cumsum(zp_cnt)])
    s_used = int(zp_off[-1])
    s_tot = _rup(max(s_used, 1), sh.stage)
    n_stages = s_tot // sh.stage
    n_mm = sh.stage // sh.bank
    n_banks = ((n_mm * n_stages - 1) // 128) + 1
    assert n_banks <= 2, (n_stages, n_banks)

    # stage call lists: walk chunks in order, splitting at stage boundaries;
    # trailing pad gathers chunk 0 row 0 (sm/mask = 0 kills it).
    zp_calls = []
    cur = []
    room = sh.stage
    for ch in range(n_zchunks):
        m = int(zp_cnt[ch])
        while m > 0:
            take = min(room, m, sh.sub)
            cur.append((ch, take))
            m -= take
            room -= take
            if room == 0:
                zp_calls.append(tuple(cur))
                cur, room = [], sh.stage
    if room < sh.stage or not zp_calls:
        while room > 0:
            take = min(room, sh.sub)
            cur.append((0, take))
            room -= take
        zp_calls.append(tuple(cur))
    assert len(zp_calls) == n_stages, (len(zp_calls), n_stages)

    plan = Plan(sh, e_pad, zv_calls, tuple(zp_calls), n_stages, n_banks, s_tot)

    # ---- per-core tensors
    in_maps = []
    ncols = plan.n_banks * sh.bank
    for c in range(sh.n_cores):
        ctx, leaf, nodes, sm, mk = cores[c]
        eord = eords[c]

        # zv: token id per (sorted) edge; int16 in-chunk ctx indices.
        zvidx = np.zeros(e_pad, np.int16)
        tok_of_edge = np.zeros(epc, np.int64)
        pos = 0
        ptr = 0  # index into eord
        for ch, off, cnt in zv_calls:
            k = int(ctx_chunk_cnt[c, ch])
            sel = eord[ptr : ptr + k]
            ptr += k
            zvidx[off : off + k] = (ctx[sel] - ch * sh.chunk).astype(np.int16)
            tok_of_edge[sel] = off + np.arange(k)
        assert ptr == epc

        # zp slots: sort by node id, bucket by chunk with per-chunk padding.
        flat_nodes = nodes.reshape(-1)
        flat_sm = sm.reshape(-1)
        flat_mk = mk.reshape(-1)
        flat_tok = tok_of_edge.repeat(L)
        sord = np.argsort(flat_nodes, kind="stable")
        zpidx = np.zeros(s_tot, np.int16)
        tokidx = np.zeros(s_tot, np.int16)
        sm_slot = np.zeros(s_tot, np.float32)
        mk_slot = np.zeros(s_tot, np.float32)
        sn = flat_nodes[sord]
        bounds = np.searchsorted(sn, np.arange(n_zchunks + 1) * sh.chunk)
        for ch in range(n_zchunks):
            lo, hi = int(bounds[ch]), int(bounds[ch + 1])
            if hi == lo and zp_cnt[ch] == 0:
                continue
            off = int(zp_off[ch])
            sel = sord[lo:hi]
            zpidx[off : off + hi - lo] = (flat_nodes[sel] - ch * sh.chunk).astype(np.int16)
            tokidx[off : off + hi - lo] = flat_tok[sel].astype(np.int16)
            sm_slot[off : off + hi - lo] = flat_sm[sel]
            mk_slot[off : off + hi - lo] = flat_mk[sel]

        # PSUM mapping: slot s -> stage t=s//stage, k=(s%stage)//bank,
        # col=s%bank; J=n_mm*t+k; bank=J//128, row=J%128.
        s_idx = np.arange(s_tot)
        t = s_idx // sh.stage
        k = (s_idx % sh.stage) // sh.bank
        col = s_idx % sh.bank
        J = n_mm * t + k
        bank_i, row = J // 128, J % 128
        sm_all = np.zeros((P, ncols), np.float32)
        mk_all = np.zeros((P, ncols), np.float32)
        sm_all[row, bank_i * sh.bank + col] = sm_slot
        mk_all[row, bank_i * sh.bank + col] = mk_slot

        in_maps.append({
            "zb": zb,
            "zvidx": _wrap_idx(zvidx),
            "zpidx": _wrap_idx(zpidx),
            "tokidx": _wrap_idx(tokidx),
            "sm_all": sm_all.astype(BF16),
            "mk_all": mk_all.astype(BF16),
        })
    return plan, in_maps


def build_kernel(tc: tile.TileContext, outs, ins, plan: Plan):
    nc = tc.nc
    sh = plan.shape
    qctr = [0]

    def next_q():
        q = qctr[0] % N_SWDGE_QUEUES
        qctr[0] += 1
        return q
    (out_d,) = outs
    zb_d, zvidx_d, zpidx_d, tokidx_d, sm_d, mk_d = ins
    f32, bf16, i16 = mybir.dt.float32, mybir.dt.bfloat16, mybir.dt.int16
    D, CH, ST, BK = sh.dim, sh.chunk, sh.stage, sh.bank
    n_mm = ST // BK  # matmuls per stage

    nc.gpsimd.load_library(library_config.mlp)

    with (
        tc.tile_pool(name="const", bufs=1) as cpool,
        tc.tile_pool(name="zp", bufs=4) as zp_pool,
        tc.tile_pool(name="zq", bufs=4) as zq_pool,
        tc.tile_pool(name="pr", bufs=4) as pr_pool,
        tc.tile_pool(name="ep", bufs=1) as ep_pool,
        tc.psum_pool(name="ps", bufs=1) as psum,
    ):
        zvidx = cpool.tile([P, plan.e_pad // 16], i16)
        zpidx = cpool.tile([P, plan.s_tot // 16], i16)
        tokidx = cpool.tile([P, plan.s_tot // 16], i16)
        ncols = plan.n_banks * BK
        sm_s = cpool.tile([P, ncols], bf16)
        mk_s = cpool.tile([P, ncols], bf16)
        nc.sync.dma_start(out=zvidx[:], in_=zvidx_d[:, :])
        nc.sync.dma_start(out=zpidx[:], in_=zpidx_d[:, :])
        nc.sync.dma_start(out=tokidx[:], in_=tokidx_d[:, :])
        nc.sync.dma_start(out=sm_s[:], in_=sm_d[:, :])
        nc.sync.dma_start(out=mk_s[:], in_=mk_d[:, :])

        # ones_buf[:, 128] = 1: lhsT slice [128-r : 256-r] puts the ones
        # column at local position r -> matmul adds colsums into PSUM row r.
        ones_buf = cpool.tile([P, 256], bf16)
        nc.vector.memset(ones_buf[:], 0.0)
        nc.vector.memset(ones_buf[:, 128:129], 1.0)

        # ---- zv: Z[ctx] token-major in SBUF (token t at partition t%128,
        # block t//128), gathered per ctx chunk.
        zv_sbuf = cpool.tile([P, plan.e_pad], bf16)
        for ch, off, cnt in plan.zv_calls:
            hi = min((ch + 1) * CH, sh.nz)
            nc.gpsimd.dma_gather(
                out_ap=zv_sbuf[:, off : off + cnt].rearrange("p (j d) -> p j d", d=D),
                in_ap=zb_d[ch * CH : hi, :],
                idxs_ap=zvidx[:, off // 16 : (off + cnt) // 16],
                num_idxs=cnt,
                num_idxs_reg=cnt,
                elem_size=D,
                queue_num=next_q(),
            )

        # ---- stages
        banks = [psum.tile([P, BK], f32, name=f"bank{b}") for b in range(plan.n_banks)]
        j_last = n_mm * plan.n_stages - 1
        for t, calls in enumerate(plan.zp_calls):
            zpT = zp_pool.tile([P, ST], bf16)
            lo = 0
            for ch, cnt in calls:
                hi = min((ch + 1) * CH, sh.nz)
                g0 = t * ST + lo
                nc.gpsimd.dma_gather(
                    out_ap=zpT[:, lo : lo + cnt].rearrange("p (u n) -> p u n", u=1),
                    in_ap=zb_d[ch * CH : hi, :],
                    idxs_ap=zpidx[:, g0 // 16 : (g0 + cnt) // 16],
                    num_idxs=cnt,
                    num_idxs_reg=cnt,
                    elem_size=D,
                    transpose=True,
                    queue_num=next_q(),
                )
                lo += cnt
            assert lo == ST
            zvT = zq_pool.tile([P, ST], bf16)
            q = 0
            while q < ST:
                qn = min(ST - q, sh.sub)
                g0 = t * ST + q
                nc.gpsimd.dma_gather(
                    out_ap=zvT[:, q : q + qn].rearrange("p (u n) -> p u n", u=1),
                    in_ap=zv_sbuf[:],
                    idxs_ap=tokidx[:, g0 // 16 : (g0 + qn) // 16],
                    num_idxs=qn,
                    num_idxs_reg=qn,
                    elem_size=D,
                    transpose=True,
                    queue_num=next_q(),
                    sbuf_tokens_per_rank=P,
                    sbuf_free_dim_per_rank=2 * D,
                )
                q += qn
            prod = pr_pool.tile([P, ST], bf16)
            nc.vector.tensor_tensor(
                out=prod[:], in0=zpT[:], in1=zvT[:], op=mybir.AluOpType.mult
            )
            for k in range(n_mm):
                j = n_mm * t + k
                b, r = j // 128, j % 128
                nc.tensor.matmul(
                    out=banks[b][:, :],
                    lhsT=ones_buf[:, 128 - r : 256 - r],
                    rhs=prod[:, k * BK : (k + 1) * BK],
                    start=(j % 128 == 0),
                    stop=(j == j_last or j % 128 == 127),
                )

        # ---- epilogue: loss_slot = mask * softplus(-sm * x)
        # softplus(-w) = relu(-w) + ln(1 + exp(-|w|))  (range-safe split)
        accs = []
        for b in range(plan.n_banks):
            w = ep_pool.tile([P, BK], f32, name="w")
            nc.vector.tensor_tensor(
                out=w[:], in0=banks[b][:, :], in1=sm_s[:, b * BK : (b + 1) * BK],
                op=mybir.AluOpType.mult,
            )
            aw = ep_pool.tile([P, BK], f32, name="aw")
            nc.scalar.activation(out=aw[:], in_=w[:], func=mybir.ActivationFunctionType.Abs)
            e2 = ep_pool.tile([P, BK], f32, name="e2")
            nc.scalar.activation(
                out=e2[:], in_=aw[:], func=mybir.ActivationFunctionType.Exp, scale=-1.0
            )
            p1 = ep_pool.tile([P, BK], f32, name="p1")
            nc.vector.scalar_tensor_tensor(
                out=p1[:], in0=e2[:], scalar=1.0, in1=e2[:],
                op0=mybir.AluOpType.add, op1=mybir.AluOpType.bypass,
            )
            lnp = ep_pool.tile([P, BK], f32, name="lnp")
            nc.scalar.activation(out=lnp[:], in_=p1[:], func=mybir.ActivationFunctionType.Ln)
            r_ = ep_pool.tile([P, BK], f32, name="r_")
            nc.scalar.activation(
                out=r_[:], in_=w[:], func=mybir.ActivationFunctionType.Relu, scale=-1.0
            )
            sp = ep_pool.tile([P, BK], f32, name="sp")
            nc.vector.tensor_tensor(out=sp[:], in0=r_[:], in1=lnp[:], op=mybir.AluOpType.add)
            junk = ep_pool.tile([P, BK], f32, name="junk")
            acc = cpool.tile([P, 1], f32, name=f"acc{b}")
            nc.vector.scalar_tensor_tensor(
                out=junk[:], in0=sp[:], scalar=0.0, in1=mk_s[:, b * BK : (b + 1) * BK],
                op0=mybir.AluOpType.add, op1=mybir.AluOpType.mult, accum_out=acc[:],
            )
            accs.append(acc)
        total = cpool.tile([P, 1], f32)
        if plan.n_banks == 2:
            nc.vector.tensor_tensor(
                out=total[:], in0=accs[0][:], in1=accs[1][:], op=mybir.AluOpType.add
            )
        else:
            nc.vector.tensor_copy(out=total[:], in_=accs[0][:])
        nc.sync.dma_start(out=out_d[:, :], in_=total[:])


def build_module(plan: Plan) -> bacc.Bacc:
    sh = plan.shape
    nc = bacc.Bacc("TRN2", target_bir_lowering=False, debug=False,
                   num_devices=sh.n_cores, num_swdge_queues=N_SWDGE_QUEUES)
    f32, bf16, i16 = mybir.dt.float32, mybir.dt.bfloat16, mybir.dt.int16
    ncols = plan.n_banks * sh.bank
    ins = [
        nc.dram_tensor("zb", [sh.nz, sh.dim], bf16, kind="ExternalInput").ap(),
        nc.dram_tensor("zvidx", [P, plan.e_pad // 16], i16, kind="ExternalInput").ap(),
        nc.dram_tensor("zpidx", [P, plan.s_tot // 16], i16, kind="ExternalInput").ap(),
        nc.dram_tensor("tokidx", [P, plan.s_tot // 16], i16, kind="ExternalInput").ap(),
        nc.dram_tensor("sm_all", [P, ncols], bf16, kind="ExternalInput").ap(),
        nc.dram_tensor("mk_all", [P, ncols], bf16, kind="ExternalInput").ap(),
    ]
    outs = [nc.dram_tensor("out", [P, 1], f32, kind="ExternalOutput").ap()]
    with tile.TileContext(nc) as tc:
        build_kernel(tc, outs, ins, plan)
    nc.compile()
    return nc


_NC_CACHE: dict = {}


def kernel(edges, path_nodes, path_signs, path_mask, Z, _results_out=None, **run_kwargs) -> np.ndarray:
    plan, in_maps = make_plan_and_maps(edges, path_nodes, path_signs, path_mask, Z)
    key = (plan.shape, plan.e_pad, plan.zv_calls, plan.zp_calls, plan.n_stages,
           N_SWDGE_QUEUES)
    if key not in _NC_CACHE:
        _NC_CACHE[key] = build_module(plan)
    nc = _NC_CACHE[key]
    res = run_bass_kernel_spmd(
        nc, in_maps, core_ids=list(range(plan.shape.n_cores)), **run_kwargs
    )
    if _results_out is not None:
        _results_out["results"] = res
    total = np.float64(0.0)
    for r in res.results:
        total += np.asarray(r["out"], dtype=np.float64).sum()
    return np.float32(total)
